# revision 1
# baseline (speedup 1.0000x reference)
"""Trainium2 Bass kernel for NeuralGraphHidden (GNN message passing).

Math (per molecule b, atom a):
    deg[b,a]    = #valid edges (edges[b,a,:] != -1)
    summed_atom = atoms[b,a] + sum_s atoms[b, edges[b,a,s]]          (64)
    bond_sum    = sum_s bonds[b,a,s]                                  (8)
    x           = concat(summed_atom, bond_sum)                      (72)
    out[b,a]    = relu(x @ Ws[deg] + bs[deg])  if deg <= 5 else 0   (128)

Design notes (driven by measured TRN2 behaviour on this system):
  * Every device-side random-row gather mechanism measured 20-500 ns/row
    (Ant dma_gather HBM ~49 ns/idx, SBUF-source ~500 ns/idx, generic
    indirect ~300 ns/row at its supported [128,1]-offset granularity, and
    wide offset APs silently corrupt data on HW).  At ~50k gathered rows
    per core that is milliseconds - 30x over the memory roofline.  The
    host therefore performs all *layout* work (degree-sort permutation,
    neighbour row expansion via np.take, bf16 packing), which is pure
    indexed data movement, and the device does all arithmetic: neighbour
    summation, transposes, per-degree dense layers, relu.
  * Pure data parallel: 128 molecules per core (8 cores), one SPMD
    program; per-degree groups padded to a fixed 2560 slots so all cores
    share it.
  * Device pipeline, all contiguous DMA:
      1. load degree-sorted token rows [atoms|bonds] (bf16, HWDGE)
      2. load expanded neighbour atom rows (bf16, HWDGE); slot-s list is
         a prefix of the degree-DESC sorted order
      3. DVE adds accumulate neighbour sums into the self rows
      4. per 128-token tile: PE transpose -> [feature, token]; one matmul
         against the tile's degree weights + a K=1 bias matmul
      5. relu on ScalarE -> bf16 sorted output rows (HWDGE store)
  * Host unpermutes the sorted output (deg-6 rows are zero).
"""

import sys

sys.path.insert(0, "/opt/trn_rl_repo")

import numpy as np
import ml_dtypes

from contextlib import ExitStack

import concourse.bacc as bacc
import concourse.tile as tile
from concourse import mybir
from concourse.bass_utils import run_bass_kernel_spmd
from concourse.masks import make_identity

# Problem shapes (hardcoded per the harness contract).
B, A, D = 1024, 128, 6
F_ATOM, F_BOND, CONV = 64, 8, 128
FAN_IN = F_ATOM + F_BOND  # 72
NCORES = 8
BS = B // NCORES          # molecules per core = 128
T = BS * A                # tokens per core = 16384
ROW = F_ATOM + D * F_BOND               # 112 features per packed row
GROUP_PAD = 2560                        # per-degree group size (static)
NSORT = D * GROUP_PAD                   # 15360 sorted slots
KT = NSORT // 128                       # 120 token tiles
KG = GROUP_PAD // 128                   # 20 tiles per degree group
# neighbour slot-s list covers sorted slots [0, PREFIX[s]) (degree-DESC)
PREFIX = [(D - 1 - s) * GROUP_PAD for s in range(D - 1)]
NCOL = [p // 128 for p in PREFIX]       # offset columns per slot: 100,80,...
SOFF = [0]
for n in NCOL:
    SOFF.append(SOFF[-1] + n)
NTOT = SOFF[-1]                         # 300 neighbour columns overall

_f32 = mybir.dt.float32
_bf16 = mybir.dt.bfloat16

_cached = {}


def build_program(repeat=1, stages="laxmr"):
    """Build the (static) per-core Bass/Tile program.

    stages: subset of l(oads) a(dds) x(transpose) m(atmul) r(elu+store)."""
    nc = bacc.Bacc("TRN2", target_bir_lowering=False, debug=False)

    xrows = nc.dram_tensor("xrows", [128, KT * ROW], _bf16,
                           kind="ExternalInput")
    nrows = nc.dram_tensor("nrows", [128, NTOT * F_ATOM], _bf16,
                           kind="ExternalInput")
    wfull = nc.dram_tensor("wfull", [D, ROW, CONV], _bf16, kind="ExternalInput")
    bsrow = nc.dram_tensor("bsrow", [D, 1, CONV], _bf16, kind="ExternalInput")
    osort = nc.dram_tensor("osort", [128, KT * CONV], _bf16,
                           kind="ExternalOutput")

    with tile.TileContext(nc) as tc, ExitStack() as ctx:
        const_pool = ctx.enter_context(tc.tile_pool(name="const", bufs=1))
        work_pool = ctx.enter_context(tc.tile_pool(name="work", bufs=1))
        xt_pool = ctx.enter_context(tc.tile_pool(name="xt", bufs=4))
        ps_pool = ctx.enter_context(tc.tile_pool(name="ps", bufs=3, space="PSUM"))
        pt_pool = ctx.enter_context(tc.tile_pool(name="pt", bufs=3, space="PSUM"))

        wfull_t, bs_t = [], []
        for d in range(D):
            wf = const_pool.tile([ROW, CONV], _bf16, tag=f"w{d}")
            nc.sync.dma_start(out=wf[:], in_=wfull[d])
            wfull_t.append(wf)
            bt = const_pool.tile([1, CONV], _bf16, tag=f"b{d}")
            nc.sync.dma_start(out=bt[:], in_=bsrow[d])
            bs_t.append(bt)
        ones = const_pool.tile([1, 128], _bf16, tag="ones")
        nc.vector.memset(ones[:], 1.0)
        ident = const_pool.tile([128, 128], _bf16, tag="ident")
        make_identity(nc, ident[:])

        for rep in range(repeat):
            # 1+2. contiguous loads (token k*128+p lives at [p, k])
            selfsb = work_pool.tile([128, KT, ROW], _bf16, tag="selfsb")
            neigh = work_pool.tile([128, NTOT, F_ATOM], _bf16, tag="neigh")
            if "l" in stages:
                nc.sync.dma_start(
                    out=selfsb[:],
                    in_=xrows[:].rearrange("p (k e) -> p k e", e=ROW))
                nc.sync.dma_start(
                    out=neigh[:],
                    in_=nrows[:].rearrange("p (k e) -> p k e", e=F_ATOM))
            elif rep == 0:
                nc.vector.memset(selfsb[:], 0.25)
                nc.vector.memset(neigh[:], 0.25)

            # 3-5. per-tile: neighbour adds, transpose, matmul, relu
            outsb = work_pool.tile([128, KT, CONV], _bf16, tag="outsb")
            if "r" not in stages and rep == 0:
                nc.vector.memset(outsb[:], 0.5)
            for k in range(KT):
                d = D - 1 - (k // KG)          # tile degree (DESC order)
                if "a" in stages:
                    for s in range(d):
                        nc.vector.tensor_add(
                            selfsb[:, k, 0:F_ATOM],
                            selfsb[:, k, 0:F_ATOM],
                            neigh[:, SOFF[s] + k, :],
                        )
                if "x" in stages:
                    pt = pt_pool.tile([ROW, 128], _bf16, tag="pt")
                    nc.tensor.transpose(out=pt[:], in_=selfsb[:, k, :],
                                        identity=ident[:])
                    xt = xt_pool.tile([ROW, 128], _bf16, tag="xtt")
                    nc.vector.tensor_copy(xt[:], pt[:])
                else:
                    xt = None
                if "m" in stages and xt is not None:
                    ps = ps_pool.tile([128, CONV], _f32, tag="ps")
                    nc.tensor.matmul(out=ps[:], lhsT=xt[:],
                                     rhs=wfull_t[d][:],
                                     start=True, stop=False)
                    nc.tensor.matmul(out=ps[:], lhsT=ones[:], rhs=bs_t[d][:],
                                     start=False, stop=True)
                    if "r" in stages:
                        nc.scalar.activation(
                            outsb[:, k, :], ps[:],
                            mybir.ActivationFunctionType.Relu)

            if "r" in stages:
                nc.sync.dma_start(
                    out=osort[:].rearrange("p (k e) -> p k e", e=CONV),
                    in_=outsb[:])

    nc.compile()
    return nc


def _get_program():
    if "nc" not in _cached:
        _cached["nc"] = build_program()
    return _cached["nc"]


def prep_core_inputs(atoms_s, bonds_s, edges_s, wfull_np, bsrow_np):
    """Host-side layout/index prep for one core's shard (numpy only)."""
    deg = (edges_s != -1).sum(axis=-1).reshape(-1)            # [T] natural
    slot_tok = np.full(NSORT, -1, np.int64)   # sorted slot -> natural token
    for d in range(D):
        toks = np.nonzero(deg == d)[0]
        n = len(toks)
        assert n <= GROUP_PAD, f"degree-{d} group has {n} > {GROUP_PAD}"
        base = (D - 1 - d) * GROUP_PAD
        slot_tok[base:base + n] = toks

    flat = np.concatenate(
        [atoms_s.reshape(T, F_ATOM), bonds_s.reshape(T, D * F_BOND)], axis=1
    ).astype(ml_dtypes.bfloat16)                              # [T, 112]
    safe = np.maximum(slot_tok, 0)
    xrows = np.where((slot_tok >= 0)[:, None], flat[safe],
                     ml_dtypes.bfloat16(0))                   # [NSORT, 112]
    # slot j -> [partition j%128, tile j//128]
    xrows = xrows.reshape(KT, 128, ROW).transpose(1, 0, 2).reshape(128, -1)

    eflat = edges_s.reshape(T, D)
    bcol = (np.arange(T) // A) * A                            # molecule base
    atoms_flat = flat[:, :F_ATOM]
    ncols = []
    for s in range(D - 1):
        slots = slot_tok[:PREFIX[s]]
        svalid = slots >= 0
        e = np.where(svalid, eflat[np.maximum(slots, 0), s], -1)
        nat = np.maximum(bcol[np.maximum(slots, 0)] + e, 0)
        rows = np.where((e >= 0)[:, None], atoms_flat[nat],
                        ml_dtypes.bfloat16(0))                # [PREFIX[s], 64]
        ncols.append(rows.reshape(NCOL[s], 128, F_ATOM))
    nrows = np.concatenate(ncols, axis=0)                     # [NTOT,128,64]
    nrows = nrows.transpose(1, 0, 2).reshape(128, -1)

    return {
        "xrows": np.ascontiguousarray(xrows),
        "nrows": np.ascontiguousarray(nrows),
        "wfull": wfull_np,
        "bsrow": bsrow_np,
    }, slot_tok


def kernel(atoms, bonds, edges, Ws, bs, trace=False):
    atoms = np.asarray(atoms)
    bonds = np.asarray(bonds)
    edges = np.asarray(edges)
    Ws = np.asarray(Ws)
    bs = np.asarray(bs)

    # Wfull rows = [Wa (64) | tile(Wb, 6) (48)]; bias via K=1 ones matmul
    wfull_np = np.zeros((D, ROW, CONV), np.float32)
    wfull_np[:, :F_ATOM] = Ws[:, :F_ATOM]
    wfull_np[:, F_ATOM:] = np.tile(Ws[:, F_ATOM:], (1, D, 1))
    wfull_np = wfull_np.astype(ml_dtypes.bfloat16)
    bsrow_np = bs.reshape(D, 1, CONV).astype(ml_dtypes.bfloat16)

    in_maps, slot_toks = [], []
    for c in range(NCORES):
        sl = slice(c * BS, (c + 1) * BS)
        m, st = prep_core_inputs(atoms[sl], bonds[sl], edges[sl],
                                 wfull_np, bsrow_np)
        in_maps.append(m)
        slot_toks.append(st)

    nc = _get_program()
    res = run_bass_kernel_spmd(nc, in_maps, core_ids=list(range(NCORES)),
                               trace=trace)
    kernel.last_results = res

    out = np.zeros((B, A, CONV), np.float32)
    for c in range(NCORES):
        osort = res.results[c]["osort"].view(ml_dtypes.bfloat16)
        osort = osort.reshape(128, KT, CONV).transpose(1, 0, 2).reshape(
            NSORT, CONV)                                      # slot-major
        st = slot_toks[c]
        real = st >= 0
        shard = out[c * BS:(c + 1) * BS].reshape(T, CONV)
        shard[st[real]] = osort[real].astype(np.float32)
    return out



# revision 2
# speedup vs baseline: 2.1971x; 2.1971x over previous
"""Trainium2 Bass kernel for NeuralGraphHidden (GNN message passing).

Math (per molecule b, atom a):
    deg[b,a]    = #valid edges (edges[b,a,:] != -1)
    summed_atom = atoms[b,a] + sum_s atoms[b, edges[b,a,s]]          (64)
    bond_sum    = sum_s bonds[b,a,s]                                  (8)
    x           = concat(summed_atom, bond_sum)                      (72)
    out[b,a]    = relu(x @ Ws[deg] + bs[deg])  if deg <= 5 else 0   (128)

Design notes (driven by measured TRN2 behaviour on this system):
  * Device-side random-row gathers measured 20-500 ns/row -> the host does
    all *layout* work (degree-sort permutation, neighbour row expansion via
    np.take, bf16 packing, feature-major transposes), which is pure indexed
    data movement; the device does all arithmetic.
  * Everything is delivered FEATURE-MAJOR (partition = feature, free = sorted
    token slot), so the device needs no transposes at all:
      - xrowsT  [112, 15360]: rows 0:64 self atom features, rows 64:112 the
        six raw bond vectors; the bond sum happens inside the matmul because
        Wb is tiled 6x along K in wfull.
      - ncatT   [64, 38400]: neighbour atom features, grouped by (degree
        group, slot s), each region [64, 2560] contiguous.
      - out[c, tok] = relu(Wd^T x + b) with conv on PARTITIONS, so the bias
        is a per-partition scalar folded into the Scalar-engine relu.
  * Per degree group d (2560 slots): neighbour slots s<3 are summed into the
    self rows by three wide DVE adds; slots s>=3 are folded into the main
    matmul via PSUM accumulation (lhsT = atom part of the weights). One
    matmul streams a 512-col quad (one PSUM bank); ~106 instructions total.
  * Host unpermutes the sorted output (deg-6 rows are zero).
"""

import sys

sys.path.insert(0, "/opt/trn_rl_repo")

import numpy as np
import ml_dtypes

from contextlib import ExitStack

import concourse.bacc as bacc
import concourse.tile as tile
from concourse import mybir
from concourse.bass_utils import run_bass_kernel_spmd

# Problem shapes (hardcoded per the harness contract).
B, A, D = 1024, 128, 6
F_ATOM, F_BOND, CONV = 64, 8, 128
FAN_IN = F_ATOM + F_BOND  # 72
NCORES = 8
BS = B // NCORES          # molecules per core = 128
T = BS * A                # tokens per core = 16384
ROW = F_ATOM + D * F_BOND               # 112 features per packed row
GROUP_PAD = 2560                        # per-degree group size (static)
NSORT = D * GROUP_PAD                   # 15360 sorted slots
QW = 512                                # quad width (one PSUM bank of f32)
NQ = GROUP_PAD // QW                    # 5 quads per group
S_DVE = 3                               # neighbour slots s < S_DVE go to DVE
# group g holds degree d = D-1-g; ncat region (g, s) starts at column
# RCOL[g] + s*GROUP_PAD
DEG = [D - 1 - g for g in range(D)]
RCOL = [0]
for g in range(D):
    RCOL.append(RCOL[-1] + DEG[g] * GROUP_PAD)
NCAT_COLS = RCOL[D]                     # 38400

_f32 = mybir.dt.float32
_bf16 = mybir.dt.bfloat16

_cached = {}


def build_program():
    """Build the (static) per-core Bass/Tile program."""
    nc = bacc.Bacc("TRN2", target_bir_lowering=False, debug=False)

    xrowsT = nc.dram_tensor("xrowsT", [ROW, NSORT], _bf16, kind="ExternalInput")
    ncatT = nc.dram_tensor("ncatT", [F_ATOM, NCAT_COLS], _bf16,
                           kind="ExternalInput")
    wfull = nc.dram_tensor("wfull", [D, ROW, CONV], _bf16, kind="ExternalInput")
    bsT = nc.dram_tensor("bsT", [CONV, D], _f32, kind="ExternalInput")
    osortT = nc.dram_tensor("osortT", [CONV, NSORT], _bf16,
                            kind="ExternalOutput")

    with tile.TileContext(nc) as tc, ExitStack() as ctx:
        const_pool = ctx.enter_context(tc.tile_pool(name="const", bufs=1))
        work_pool = ctx.enter_context(tc.tile_pool(name="work", bufs=1))
        ps_pool = ctx.enter_context(tc.tile_pool(name="ps", bufs=8,
                                                 space="PSUM"))

        wfull_t = []
        for d in range(D):
            wf = const_pool.tile([ROW, CONV], _bf16, tag=f"w{d}")
            nc.sync.dma_start(out=wf[:], in_=wfull[d])
            wfull_t.append(wf)
        bs_t = const_pool.tile([CONV, D], _f32, tag="bsT")
        nc.sync.dma_start(out=bs_t[:], in_=bsT[:])

        # Per-group input loads, lightest degree first (pipeline ramp).
        gorder = list(range(D - 1, -1, -1))        # g=5 (d=0) ... g=0 (d=5)
        xt_t, nc_t = {}, {}
        for g in gorder:
            d = DEG[g]
            xt = work_pool.tile([ROW, GROUP_PAD], _bf16, tag=f"xt{g}")
            nc.sync.dma_start(
                out=xt[:], in_=xrowsT[:, g * GROUP_PAD:(g + 1) * GROUP_PAD])
            xt_t[g] = xt
            if d > 0:
                nct = work_pool.tile([F_ATOM, d * GROUP_PAD], _bf16,
                                     tag=f"nc{g}")
                nc.sync.dma_start(
                    out=nct[:], in_=ncatT[:, RCOL[g]:RCOL[g + 1]])
                nc_t[g] = nct

        for g in gorder:
            d = DEG[g]
            xt = xt_t[g]
            # neighbour slots s < S_DVE: wide in-place adds on DVE
            for s in range(min(d, S_DVE)):
                nc.vector.tensor_add(
                    xt[0:F_ATOM, :],
                    xt[0:F_ATOM, :],
                    nc_t[g][:, s * GROUP_PAD:(s + 1) * GROUP_PAD],
                )
            out_g = work_pool.tile([CONV, GROUP_PAD], _bf16, tag=f"out{g}")
            n_pe = max(0, d - S_DVE)
            for q in range(NQ):
                cols = slice(q * QW, (q + 1) * QW)
                ps = ps_pool.tile([CONV, QW], _f32, tag="ps")
                nc.tensor.matmul(out=ps[:], lhsT=wfull_t[d][:],
                                 rhs=xt[:, cols],
                                 start=True, stop=(n_pe == 0))
                for j, s in enumerate(range(S_DVE, d)):
                    nc.tensor.matmul(
                        out=ps[:], lhsT=wfull_t[d][0:F_ATOM, :],
                        rhs=nc_t[g][:, s * GROUP_PAD + q * QW:
                                    s * GROUP_PAD + (q + 1) * QW],
                        start=False, stop=(j == n_pe - 1))
                nc.scalar.activation(out_g[:, cols], ps[:],
                                     mybir.ActivationFunctionType.Relu,
                                     bias=bs_t[:, d:d + 1])
            nc.sync.dma_start(
                out=osortT[:, g * GROUP_PAD:(g + 1) * GROUP_PAD],
                in_=out_g[:])

    nc.compile()
    return nc


def _get_program():
    if "nc" not in _cached:
        _cached["nc"] = build_program()
    return _cached["nc"]


def prep_core_inputs(atoms_s, bonds_s, edges_s, wfull_np, bsT_np):
    """Host-side layout/index prep for one core's shard (numpy only)."""
    deg = (edges_s != -1).sum(axis=-1).reshape(-1)            # [T] natural
    slot_tok = np.full(NSORT, -1, np.int64)   # sorted slot -> natural token
    for d in range(D):
        toks = np.nonzero(deg == d)[0]
        n = len(toks)
        assert n <= GROUP_PAD, f"degree-{d} group has {n} > {GROUP_PAD}"
        base = (D - 1 - d) * GROUP_PAD
        slot_tok[base:base + n] = toks

    flat = np.concatenate(
        [atoms_s.reshape(T, F_ATOM), bonds_s.reshape(T, D * F_BOND)], axis=1
    ).astype(ml_dtypes.bfloat16)                              # [T, 112]
    safe = np.maximum(slot_tok, 0)
    xrows = np.where((slot_tok >= 0)[:, None], flat[safe],
                     ml_dtypes.bfloat16(0))                   # [NSORT, 112]
    xrowsT = np.ascontiguousarray(xrows.T)                    # [112, NSORT]

    eflat = edges_s.reshape(T, D)
    bcol = (np.arange(T) // A) * A                            # molecule base
    atoms_flat = flat[:, :F_ATOM]
    regions = []
    for g in range(D):
        d = DEG[g]
        slots = slot_tok[g * GROUP_PAD:(g + 1) * GROUP_PAD]
        sv = slots >= 0
        st = np.maximum(slots, 0)
        for s in range(d):
            e = np.where(sv, eflat[st, s], -1)
            nat = np.maximum(bcol[st] + e, 0)
            regions.append(np.where((e >= 0)[:, None], atoms_flat[nat],
                                    ml_dtypes.bfloat16(0)))   # [2560, 64]
    ncat = np.concatenate(regions, axis=0)                    # [38400, 64]
    ncatT = np.ascontiguousarray(ncat.T)                      # [64, 38400]

    return {
        "xrowsT": xrowsT,
        "ncatT": ncatT,
        "wfull": wfull_np,
        "bsT": bsT_np,
    }, slot_tok


def kernel(atoms, bonds, edges, Ws, bs, trace=False):
    atoms = np.asarray(atoms)
    bonds = np.asarray(bonds)
    edges = np.asarray(edges)
    Ws = np.asarray(Ws)
    bs = np.asarray(bs)

    # Wfull rows = [Wa (64) | tile(Wb, 6) (48)]; the 6x tiling makes the
    # matmul itself perform the bond sum.
    wfull_np = np.zeros((D, ROW, CONV), np.float32)
    wfull_np[:, :F_ATOM] = Ws[:, :F_ATOM]
    wfull_np[:, F_ATOM:] = np.tile(Ws[:, F_ATOM:], (1, D, 1))
    wfull_np = wfull_np.astype(ml_dtypes.bfloat16)
    bsT_np = np.ascontiguousarray(bs.T.astype(np.float32))    # [128, 6]

    in_maps, slot_toks = [], []
    for c in range(NCORES):
        sl = slice(c * BS, (c + 1) * BS)
        m, st = prep_core_inputs(atoms[sl], bonds[sl], edges[sl],
                                 wfull_np, bsT_np)
        in_maps.append(m)
        slot_toks.append(st)

    nc = _get_program()
    res = run_bass_kernel_spmd(nc, in_maps, core_ids=list(range(NCORES)),
                               trace=trace)
    kernel.last_results = res

    out = np.zeros((B, A, CONV), np.float32)
    for c in range(NCORES):
        osortT = res.results[c]["osortT"].view(ml_dtypes.bfloat16)
        osort = osortT.reshape(CONV, NSORT).T                 # [NSORT, 128]
        st = slot_toks[c]
        real = st >= 0
        shard = out[c * BS:(c + 1) * BS].reshape(T, CONV)
        shard[st[real]] = osort[real].astype(np.float32)
    return out
